# revision 1
# baseline (speedup 1.0000x reference)
"""Trainium2 Bass kernel for nn_MultiHeadLinearAttention (Linformer attention).

B=4, T=4096, C=1024, H=16, HS=64, K=256.
Sharding: 8 cores = batch (4) x head-group (2 groups of 8 heads).
Per core: qkv projections, low-rank kpT/vpT = k^T E / v^T E accumulated over
T, masked softmax attention over the compressed dim (exp on ScalarE with
fused row-sum, PE transposes), and a column-shard of the output projection.
Host sums the two partial projections per batch.

Matmul inputs on the q/k/v/E/softmax path are bf16; the final projection
stays fp32r. Odd heads pack kv as [v|k] so kpT lands in partitions 64:128,
letting S-matmuls run K=128 against zero-padded kp pair tiles with
full-partition qT tiles (no cross-partition moves, no DRAM spill of qT);
one N=512 S-matmul computes both heads of a pair.
"""
import sys
for p in ('/opt/trn_rl_repo', '/root/.axon_site/_ro/trn_rl_repo'):
    if p not in sys.path:
        sys.path.insert(0, p)

from contextlib import ExitStack

import numpy as np

import concourse.bacc as bacc
import concourse.mybir as mybir
from concourse import tile
from concourse.bass_utils import run_bass_kernel_spmd

f32 = mybir.dt.float32
f32r = mybir.dt.float32r
bf16 = mybir.dt.bfloat16
AF = mybir.ActivationFunctionType

B, T, C = 4, 4096, 1024
H, HS = 16, 64
K = 256
HL = 8            # heads per core
TB = 512          # t-block
NTB = T // TB     # 8
NC_ = C // 128    # 8 c-chunks
SCALE = 1.0 / np.sqrt(np.float32(K))  # 1/16


def to_f32r(a: np.ndarray) -> np.ndarray:
    """Round fp32 -> fp32r bit format (11-bit mantissa, low 12 bits zero), RNE."""
    b = np.ascontiguousarray(a, dtype=np.float32).view(np.uint32)
    add = np.uint32(0x7FF) + ((b >> np.uint32(12)) & np.uint32(1))
    return ((b + add) & np.uint32(0xFFFFF000)).view(np.float32)


def _build_program(phases=3, repeat=1, timing=False):
    nc = bacc.Bacc("TRN2", target_bir_lowering=False, debug=False, num_devices=8)

    if timing:
        DIN = nc.declare_dram_parameter("DIN", [128, 128], f32, isOutput=False)
        DOUT = nc.declare_dram_parameter("DOUT", [128, 128], f32, isOutput=True)
        decl = lambda name, shape, dt_, out=False: nc.dram_tensor(name, shape, dt_)
    else:
        decl = lambda name, shape, dt_, out=False: nc.declare_dram_parameter(
            name, shape, dt_, isOutput=out)
    XT = decl("XT", [C, T], bf16)
    WQ = decl("WQ", [128, NC_ * 512], bf16)   # host-packed: chunk c at cols c*512
    WK = decl("WK", [128, NC_ * 512], bf16)
    WV = decl("WV", [128, NC_ * 512], bf16)
    ED = decl("ED", [HL, T, K], bf16)
    WPT = decl("WPT", [128, 4 * C], f32r)     # host-packed: ci-chunk m at cols m*C
    MSK = decl("MSK", [2, 128, K], f32)
    IDN = decl("IDN", [128, 128], f32)
    O = decl("O", [T, C], f32, out=True)

    with tile.TileContext(nc) as tc, ExitStack() as top:
        # ---- persistent pools ----
        misc = top.enter_context(tc.tile_pool(name="misc", bufs=1))
        kvacc_p = top.enter_context(tc.tile_pool(name="kvacc", bufs=1))
        vp_p = top.enter_context(tc.tile_pool(name="vp", bufs=1))
        qres_p = top.enter_context(tc.tile_pool(name="qres", bufs=1))

        identf = misc.tile([128, 128], f32, tag="identf", name="identf")
        nc.sync.dma_start(identf[:], IDN[:])
        ident = misc.tile([128, 128], f32r, tag="ident", name="ident")
        nc.vector.tensor_copy(ident[:].bitcast(f32r), identf[:])
        identb = misc.tile([128, 128], bf16, tag="identb", name="identb")
        nc.scalar.copy(identb[:], identf[:])
        masksb = []
        for i in range(2):
            mt = misc.tile([128, K], f32, tag=f"msk{i}", name=f"msk{i}")
            nc.sync.dma_start(mt[:], MSK[i])
            masksb.append(mt)

        if phases:
            kvacc = [kvacc_p.tile([128, K], f32, tag=f"kvacc{h}", name=f"kvacc{h}")
                     for h in range(HL)]
            vp_sb = [vp_p.tile([128, 128], bf16, tag=f"vp{h}", name=f"vp{h}")
                     for h in range(HL)]
            kpbp = [vp_p.tile([128, 2 * K], bf16, tag=f"kpbp{pr}", name=f"kpbp{pr}")
                    for pr in range(4)]
            qres = [qres_p.tile([128, T], bf16, tag=f"qres{m}", name=f"qres{m}")
                    for m in range(4)]

        for _rep in range(max(1, repeat)):
            # ================= PHASE 1 =================
            if phases & 1:
                with ExitStack() as s1:
                    w_p = s1.enter_context(tc.tile_pool(name="w", bufs=1))
                    xt_p = s1.enter_context(tc.tile_pool(name="xt", bufs=10))
                    e_p = s1.enter_context(tc.tile_pool(name="e", bufs=10))
                    kv_p = s1.enter_context(tc.tile_pool(name="kv", bufs=2))
                    psq_p = s1.enter_context(tc.tile_pool(name="psq", bufs=2, space="PSUM"))
                    psk_p = s1.enter_context(tc.tile_pool(name="psk", bufs=2, space="PSUM"))
                    psv_p = s1.enter_context(tc.tile_pool(name="psv", bufs=2, space="PSUM"))
                    pskv_p = s1.enter_context(tc.tile_pool(name="pskv", bufs=2, space="PSUM"))

                    wqt = w_p.tile([128, NC_ * 512], bf16, tag="wqt", name="wqt")
                    wkt = w_p.tile([128, NC_ * 512], bf16, tag="wkt", name="wkt")
                    wvt = w_p.tile([128, NC_ * 512], bf16, tag="wvt", name="wvt")
                    nc.sync.dma_start(wqt[:], WQ[:])
                    nc.sync.dma_start(wkt[:], WK[:])
                    nc.sync.dma_start(wvt[:], WV[:])

                    xtt = [None] * NC_
                    ett = [None] * HL
                    for tb in range(NTB):
                        t0 = tb * TB
                        tbo = tb % 2
                        if tbo == 0:
                            for c in range(NC_):
                                x_t = xt_p.tile([128, 2 * TB], bf16, tag="xt", name="xt")
                                nc.sync.dma_start(x_t[:], XT[c * 128:(c + 1) * 128,
                                                             t0:t0 + 2 * TB])
                                xtt[c] = x_t
                            for h in range(HL):
                                e_t = e_p.tile([128, 8, K], bf16, tag="et", name="et")
                                src = ED[h, t0:t0 + 2 * TB, :].rearrange(
                                    "(s p) r -> p s r", p=128)
                                nc.sync.dma_start(e_t[:], src)
                                ett[h] = e_t

                        # Q projection -> resident qT (bf16, head-major rows)
                        for m in range(4):
                            psq = psq_p.tile([128, 512], f32, tag="psq", name="psq")
                            for c in range(NC_):
                                nc.tensor.matmul(psq[:],
                                                 wqt[:, c * 512 + m * 128:c * 512 + (m + 1) * 128],
                                                 xtt[c][:, tbo * TB:(tbo + 1) * TB],
                                                 start=(c == 0), stop=(c == NC_ - 1))
                            if m % 2 == 0:
                                nc.scalar.copy(qres[m][:, t0:t0 + TB], psq[:])
                            else:
                                nc.vector.tensor_copy(qres[m][:, t0:t0 + TB], psq[:])

                        # K,V projections -> packed kv tiles:
                        # even head h: cols [h*128: k(64) | v(64)], odd head: [v | k]
                        kvsb = []
                        for sub in range(4):
                            psk = psk_p.tile([128, 512], f32, tag="psk", name="psk")
                            psv = psv_p.tile([128, 512], f32, tag="psv", name="psv")
                            for c in range(NC_):
                                nc.tensor.matmul(psk[:],
                                                 xtt[c][:, tbo * TB + sub * 128:tbo * TB + (sub + 1) * 128],
                                                 wkt[:, c * 512:(c + 1) * 512],
                                                 start=(c == 0), stop=(c == NC_ - 1))
                            for c in range(NC_):
                                nc.tensor.matmul(psv[:],
                                                 xtt[c][:, tbo * TB + sub * 128:tbo * TB + (sub + 1) * 128],
                                                 wvt[:, c * 512:(c + 1) * 512],
                                                 start=(c == 0), stop=(c == NC_ - 1))
                            kvt = kv_p.tile([128, 1024], bf16, tag=f"kv{sub}", name=f"kv{sub}")
                            kv4 = kvt[:].rearrange("p (hp x s) -> p hp x s", hp=4, x=4, s=HS)
                            psk4 = psk[:].rearrange("p (hp e s) -> p hp e s", hp=4, e=2, s=HS)
                            psv4 = psv[:].rearrange("p (hp e s) -> p hp e s", hp=4, e=2, s=HS)
                            nc.scalar.copy(kv4[:, :, 0, :], psk4[:, :, 0, :])
                            nc.vector.tensor_copy(kv4[:, :, 1, :], psv4[:, :, 0, :])
                            nc.scalar.copy(kv4[:, :, 2, :], psv4[:, :, 1, :])
                            nc.vector.tensor_copy(kv4[:, :, 3, :], psk4[:, :, 1, :])
                            kvsb.append(kvt)

                        # kpT/vpT accumulation; even h -> rows [kpT; vpT], odd -> [vpT; kpT]
                        for h in range(HL):
                            pskv = pskv_p.tile([128, K], f32, tag="pskv", name="pskv")
                            for sub in range(4):
                                nc.tensor.matmul(pskv[:], kvsb[sub][:, h * 128:(h + 1) * 128],
                                                 ett[h][:, tbo * 4 + sub, :],
                                                 start=(sub == 0), stop=(sub == 3))
                            if tb == 0:
                                nc.vector.tensor_copy(kvacc[h][:].bitcast(f32r), pskv[:])
                            else:
                                nc.vector.tensor_tensor(kvacc[h][:].bitcast(f32r), kvacc[h][:],
                                                        pskv[:], op=mybir.AluOpType.add)

                    # phase 1.5: vp via transpose; kp pair tiles (bf16, zero-padded)
                    for h in range(HL):
                        pr, h01 = h // 2, h % 2
                        lo = (h01 == 0)   # kpT rows 0:64 if even head, else 64:128
                        half = kpbp[pr][:, h01 * K:(h01 + 1) * K]
                        if lo:
                            nc.gpsimd.memset(half[HS:128, :], 0.0)
                            nc.scalar.copy(half[0:HS, :], kvacc[h][0:HS, :])
                        else:
                            nc.gpsimd.memset(half[0:HS, :], 0.0)
                            nc.scalar.copy(half[HS:128, :], kvacc[h][HS:128, :])
                        for j in range(2):
                            psvp = pskv_p.tile([128, K], f32, tag="pskv", name="psvp")
                            nc.tensor.transpose(psvp[:, 0:128].bitcast(f32r),
                                                kvacc[h][:, j * 128:(j + 1) * 128].bitcast(f32r),
                                                ident[:])
                            vcols = psvp[:, 64:128] if lo else psvp[:, 0:64]
                            if j == 0:
                                nc.scalar.copy(vp_sb[h][:, 0:HS], vcols)
                            else:
                                nc.vector.tensor_copy(vp_sb[h][:, HS:2 * HS], vcols)

            # ================= PHASE 2 =================
            if phases & 2:
                with ExitStack() as s2:
                    wp_p = s2.enter_context(tc.tile_pool(name="wp", bufs=1))
                    ew_p = s2.enter_context(tc.tile_pool(name="ew", bufs=6))
                    wn_p = s2.enter_context(tc.tile_pool(name="wn", bufs=6))
                    zz_p = s2.enter_context(tc.tile_pool(name="zz", bufs=6))
                    wt_p = s2.enter_context(tc.tile_pool(name="wt", bufs=2))
                    at_p = s2.enter_context(tc.tile_pool(name="at", bufs=2))
                    atmp_p = s2.enter_context(tc.tile_pool(name="atmp", bufs=2))
                    out_p = s2.enter_context(tc.tile_pool(name="outp", bufs=3))
                    pss_p = s2.enter_context(tc.tile_pool(name="pss", bufs=2, space="PSUM"))
                    pswt_p = s2.enter_context(tc.tile_pool(name="pswt", bufs=2, space="PSUM"))
                    pso_p = s2.enter_context(tc.tile_pool(name="pso", bufs=1, space="PSUM"))
                    psp_p = s2.enter_context(tc.tile_pool(name="psp", bufs=1, space="PSUM"))

                    wpt = wp_p.tile([128, 4 * C], f32r, tag="wpt", name="wpt")
                    nc.sync.dma_start(wpt[:], WPT[:])

                    for tb in range(NTB):
                        t0 = tb * TB
                        attT = [at_p.tile([128, TB], f32r, tag=f"attT{p}", name=f"attT{p}")
                                for p in range(4)]
                        for pr in range(4):
                            pswt = [None, None]
                            for h01 in range(2):
                                pswt[h01] = pswt_p.tile([128, TB], f32,
                                                        tag=f"pswt{h01}", name=f"pswt{h01}")
                            wns = [[None] * 4, [None] * 4]
                            for sub in range(4):
                                tt = tb * 4 + sub
                                pss = pss_p.tile([128, 2 * K], f32, tag="pss", name="pss")
                                nc.tensor.matmul(pss[:],
                                                 qres[pr][:, tt * 128:(tt + 1) * 128],
                                                 kpbp[pr][:], start=True, stop=True)
                                for h01 in range(2):
                                    half = pss[:, h01 * K:(h01 + 1) * K]
                                    if tt < 2:
                                        nc.vector.tensor_tensor(half, half, masksb[tt][:],
                                                                op=mybir.AluOpType.add)
                                    expw = ew_p.tile([128, K], f32, tag="expw", name="expw")
                                    z = zz_p.tile([128, 1], f32, tag="z", name="z")
                                    nc.scalar.activation(expw[:], half, AF.Exp,
                                                         scale=float(SCALE), accum_out=z[:])
                                    rec = zz_p.tile([128, 1], f32, tag="rec", name="rec")
                                    nc.vector.reciprocal(rec[:], z[:])
                                    wn = wn_p.tile([128, K], bf16, tag="wn", name="wn")
                                    nc.vector.tensor_scalar_mul(wn[:], expw[:], rec[:])
                                    nc.tensor.transpose(
                                        pswt[h01][:].bitcast(bf16)[:, sub * 128:(sub + 1) * 128],
                                        wn[:, 0:128], identb[:])
                                    nc.tensor.transpose(
                                        pswt[h01][:].bitcast(bf16)[:, TB + sub * 128:TB + (sub + 1) * 128],
                                        wn[:, 128:256], identb[:])
                            for h01 in range(2):
                                h = 2 * pr + h01
                                wt0 = wt_p.tile([128, TB], bf16, tag="wt0", name="wt0")
                                wt1 = wt_p.tile([128, TB], bf16, tag="wt1", name="wt1")
                                nc.vector.tensor_copy(wt0[:], pswt[h01][:].bitcast(bf16)[:, 0:TB])
                                nc.vector.tensor_copy(wt1[:], pswt[h01][:].bitcast(bf16)[:, TB:2 * TB])
                                pso = pso_p.tile([HS, TB], f32, tag="pso", name="pso")
                                nc.tensor.matmul(pso[:], vp_sb[h][:, 0:HS], wt0[:],
                                                 start=True, stop=False)
                                nc.tensor.matmul(pso[:], vp_sb[h][:, HS:2 * HS], wt1[:],
                                                 start=False, stop=True)
                                if h01 == 0:
                                    nc.vector.tensor_copy(attT[pr][0:HS, :], pso[:])
                                else:
                                    atmp = atmp_p.tile([HS, TB], f32r, tag="atmp", name="atmp")
                                    nc.scalar.copy(atmp[:], pso[:])
                                    nc.sync.dma_start(attT[pr][HS:128, :], atmp[:])

                        # output projection for this t-block
                        for sub in range(4):
                            outsb = out_p.tile([128, C], f32, tag="outsb", name="outsb")
                            for n in range(2):
                                psp = psp_p.tile([128, 512], f32, tag="psp", name="psp")
                                for ci in range(4):
                                    nc.tensor.matmul(psp[:],
                                                     attT[ci][:, sub * 128:(sub + 1) * 128],
                                                     wpt[:, ci * C + n * 512:ci * C + (n + 1) * 512],
                                                     start=(ci == 0), stop=(ci == 3))
                                if n == 0:
                                    nc.scalar.copy(outsb[:, 0:512], psp[:])
                                else:
                                    nc.vector.tensor_copy(outsb[:, 512:1024], psp[:])
                            row = t0 + sub * 128
                            nc.sync.dma_start(O[row:row + 128, :], outsb[:])

        if timing:
            dpool = top.enter_context(tc.tile_pool(name="dummy", bufs=1))
            dt_ = dpool.tile([128, 128], f32, tag="dummy", name="dummy")
            nc.sync.dma_start(dt_[:], DIN[:])
            nc.sync.dma_start(DOUT[:], dt_[:])

    nc.finalize()
    return nc


_NC_CACHE = {}


def _get_program(phases=3):
    if phases not in _NC_CACHE:
        _NC_CACHE[phases] = _build_program(phases)
    return _NC_CACHE[phases]


def _pack_w(w_core):
    """[C, 512] -> [128, 8*512] with chunk c at cols c*512."""
    return np.ascontiguousarray(
        w_core.reshape(NC_, 128, 512).transpose(1, 0, 2).reshape(128, NC_ * 512))


def _make_in_maps(x, WQ, WK, WV, E, Wp):
    import ml_dtypes
    xr = np.transpose(np.asarray(x), (0, 2, 1)).astype(ml_dtypes.bfloat16)  # [B, C, T]
    wq_full = np.transpose(np.asarray(WQ), (1, 0, 2)).astype(ml_dtypes.bfloat16)
    wk_full = np.transpose(np.asarray(WK), (1, 0, 2)).astype(ml_dtypes.bfloat16)
    wv_full = np.transpose(np.asarray(WV), (1, 0, 2)).astype(ml_dtypes.bfloat16)
    er = np.asarray(E).astype(ml_dtypes.bfloat16)                 # [H, B, T, K]
    wpt_full = to_f32r(np.ascontiguousarray(np.asarray(Wp).T))    # [C_in, C_out]

    msk = np.zeros((2, 128, K), np.float32)
    for i in range(2):
        t_idx = i * 128 + np.arange(128)[:, None]
        msk[i] = np.where(np.arange(K)[None, :] <= t_idx, 0.0, -1e30)
    idn = np.eye(128, dtype=np.float32)

    in_maps = []
    for core in range(8):
        b, g = core // 2, core % 2
        hs = slice(g * HL, (g + 1) * HL)
        wpt_core = wpt_full[g * 512:(g + 1) * 512, :]              # [512, 1024]
        wpt_packed = np.ascontiguousarray(
            wpt_core.reshape(4, 128, C).transpose(1, 0, 2).reshape(128, 4 * C))
        in_maps.append({
            "XT": np.ascontiguousarray(xr[b]),
            "WQ": _pack_w(np.ascontiguousarray(wq_full[:, hs, :]).reshape(C, HL * HS)),
            "WK": _pack_w(np.ascontiguousarray(wk_full[:, hs, :]).reshape(C, HL * HS)),
            "WV": _pack_w(np.ascontiguousarray(wv_full[:, hs, :]).reshape(C, HL * HS)),
            "ED": np.ascontiguousarray(er[hs, b]),
            "WPT": wpt_packed,
            "MSK": msk,
            "IDN": idn,
        })
    return in_maps


def _run(x, WQ, WK, WV, E, Wp, bp, trace=False):
    nc = _get_program()
    in_maps = _make_in_maps(x, WQ, WK, WV, E, Wp)
    kw = {}
    if trace:
        kw = dict(trace=True, trace_cores=[0])
    res = run_bass_kernel_spmd(nc, in_maps, list(range(8)), **kw)
    out = np.zeros((B, T, C), np.float32)
    for b in range(B):
        out[b] = res.results[2 * b]["O"] + res.results[2 * b + 1]["O"]
    out += np.asarray(bp, np.float32)[None, None, :]
    return out, res


def kernel(x, WQ, WK, WV, E, Wp, bp):
    out, _ = _run(x, WQ, WK, WV, E, Wp, bp, trace=False)
    return out


def kernel_traced(x, WQ, WK, WV, E, Wp, bp):
    out, res = _run(x, WQ, WK, WV, E, Wp, bp, trace=True)
    return out, res



# revision 16
# speedup vs baseline: 108.8355x; 108.8355x over previous
"""Trainium2 Bass kernel for nn_MultiHeadLinearAttention (Linformer attention).

B=4, T=4096, C=1024, H=16, HS=64, K=256.
Sharding: 8 cores = batch (4) x head-group (2 groups of 8 heads).
Per core: qkv projections, low-rank kpT/vpT = k^T E / v^T E accumulated over
T, masked softmax attention over the compressed dim, and a column-shard of
the output projection. Host sums the two partial projections per batch.

v2 changes over the 433.7us baseline:
- Q/K projections run fp8(e4m3) DoubleRow (2 c-chunks per pass). Weights are
  host-scaled by 64 into e4m3's normal range; the 64*64 factor on S is folded
  into the softmax exp scale. V path stays bf16 (fp8 there fails tolerance).
- Phase 2 computes S^T (compressed dim r on partitions) so exp output feeds
  the attention matmul directly -- no per-tile PE transposes of softmax
  weights. z rides as a ones-column on even heads' att matmul; odd heads use
  a 1-row ones matmul. 1/z is broadcast across partitions on GpSimd
  (partition_broadcast) and applied on DVE while copying PSUM->attT.
- DMAs batched per 2-t-block group (x8/xtt/E in 1-2 DMAs each) and ordered
  so Q's operands land first; wpt prefetched during phase 1.
"""
import sys
for p in ('/opt/trn_rl_repo', '/root/.axon_site/_ro/trn_rl_repo'):
    if p not in sys.path:
        sys.path.insert(0, p)

from contextlib import ExitStack

import numpy as np

import concourse.bacc as bacc
import concourse.mybir as mybir
from concourse import tile
from concourse.bass_utils import run_bass_kernel_spmd

f32 = mybir.dt.float32
f32r = mybir.dt.float32r
bf16 = mybir.dt.bfloat16
fp8 = mybir.dt.float8e4
AF = mybir.ActivationFunctionType
DR = mybir.MatmulPerfMode.DoubleRow

B, T, C = 4, 4096, 1024
H, HS = 16, 64
K = 256
HL = 8            # heads per core
TB = 512          # t-block
NTB = T // TB     # 8
NC_ = C // 128    # 8 c-chunks
WS = 64.0         # fp8 weight pre-scale (host)
SCALE = 1.0 / (np.sqrt(np.float32(K)) * WS * WS)  # folded exp scale


def to_f32r(a: np.ndarray) -> np.ndarray:
    """Round fp32 -> fp32r bit format (11-bit mantissa, low 12 bits zero), RNE."""
    b = np.ascontiguousarray(a, dtype=np.float32).view(np.uint32)
    add = np.uint32(0x7FF) + ((b >> np.uint32(12)) & np.uint32(1))
    return ((b + add) & np.uint32(0xFFFFF000)).view(np.float32)


def _build_program(phases=3, repeat=1, timing=False):
    nc = bacc.Bacc("TRN2", target_bir_lowering=False, debug=False, num_devices=8)

    if timing:
        DIN = nc.declare_dram_parameter("DIN", [128, 128], f32, isOutput=False)
        DOUT = nc.declare_dram_parameter("DOUT", [128, 128], f32, isOutput=True)
        decl = lambda name, shape, dt_, out=False: nc.dram_tensor(name, shape, dt_)
    else:
        decl = lambda name, shape, dt_, out=False: nc.declare_dram_parameter(
            name, shape, dt_, isOutput=out)
    X8 = decl("X8", [C, T], fp8)              # fp8 x^T (for Q,K)
    XT = decl("XT", [C, T], bf16)             # bf16 x^T (for V)
    WQ8 = decl("WQ8", [128, 4096], fp8)       # [p, g(4), i(2), m(512)] pairs
    WK8 = decl("WK8", [128, 4096], fp8)
    WV = decl("WV", [128, NC_ * 512], bf16)   # host-packed: chunk c at cols c*512
    # host-pre-transposed E: col block ((tb*HL + h)*4 + s)*K holds the
    # [128, K] chunk for (tb, head h, sub s)
    ED = decl("ED", [128, NTB * HL * 4 * K], bf16)
    WPT = decl("WPT", [128, 4 * C], f32r)     # host-packed: ci-chunk m at cols m*C
    MSKT = decl("MSKT", [2, 128, TB], f32)    # transposed masks (tb=0 only)
    IDN = decl("IDN", [128, 128], f32)
    O = decl("O", [T, C], f32, out=True)

    with tile.TileContext(nc) as tc, ExitStack() as top:
        # ---- persistent pools ----
        misc = top.enter_context(tc.tile_pool(name="misc", bufs=1))
        kvacc_p = top.enter_context(tc.tile_pool(name="kvacc", bufs=1))
        kpb_p = top.enter_context(tc.tile_pool(name="kpb", bufs=1))
        vpz_p = top.enter_context(tc.tile_pool(name="vpz", bufs=1))
        qres_p = top.enter_context(tc.tile_pool(name="qres", bufs=1))
        wp_p = top.enter_context(tc.tile_pool(name="wp", bufs=1))

        identf = misc.tile([128, 128], f32, tag="identf", name="identf")
        nc.sync.dma_start(identf[:], IDN[:])
        ident = misc.tile([128, 128], f32r, tag="ident", name="ident")
        nc.vector.tensor_copy(ident[:].bitcast(f32r), identf[:])
        # z-selector constants: zsel[0] = [ones x 64 cols | zeros], zsel[1] inverse
        zsel = []
        for i in range(2):
            zt = misc.tile([128, 128], bf16, tag=f"zsel{i}", name=f"zsel{i}")
            nc.gpsimd.memset(zt[:, 0:64], 1.0 if i == 0 else 0.0)
            nc.gpsimd.memset(zt[:, 64:128], 0.0 if i == 0 else 1.0)
            zsel.append(zt)
        masksT = []
        for i in range(2):
            mt = misc.tile([128, TB], f32, tag=f"mskT{i}", name=f"mskT{i}")
            nc.sync.dma_start(mt[:], MSKT[i])
            masksT.append(mt)

        if phases:
            kvacc = [kvacc_p.tile([128, K], f32, tag=f"kvacc{h}", name=f"kvacc{h}")
                     for h in range(HL)]
            # kpb[h]: bf16 kpT in its head-half rows, other half zeroed
            kpb = [kpb_p.tile([128, K], bf16, tag=f"kpb{h}", name=f"kpb{h}")
                   for h in range(HL)]
            # vpz[h]: [128, 2, 128] per j-chunk [vp_e | 0] (even) / [0 | vp_o] (odd)
            vpz = [vpz_p.tile([128, 2, 128], bf16, tag=f"vpz{h}", name=f"vpz{h}")
                   for h in range(HL)]
            qres = [qres_p.tile([128, T], bf16, tag=f"qres{m}", name=f"qres{m}")
                    for m in range(4)]
        wpt = wp_p.tile([128, 4 * C], f32r, tag="wpt", name="wpt")

        for _rep in range(max(1, repeat)):
            # ================= PHASE 1 =================
            if phases & 1:
                with ExitStack() as s1:
                    w_p = s1.enter_context(tc.tile_pool(name="w", bufs=1))
                    x8_p = s1.enter_context(tc.tile_pool(name="x8", bufs=2))
                    xt_p = s1.enter_context(tc.tile_pool(name="xt", bufs=2))
                    e_p = s1.enter_context(tc.tile_pool(name="e", bufs=2))
                    kv_p = s1.enter_context(tc.tile_pool(name="kv", bufs=2))
                    psq_p = s1.enter_context(tc.tile_pool(name="psq", bufs=2, space="PSUM"))
                    psk_p = s1.enter_context(tc.tile_pool(name="psk", bufs=2, space="PSUM"))
                    psv_p = s1.enter_context(tc.tile_pool(name="psv", bufs=2, space="PSUM"))
                    pskv_p = s1.enter_context(tc.tile_pool(name="pskv", bufs=2, space="PSUM"))

                    wq8 = w_p.tile([128, 4096], fp8, tag="wq8", name="wq8")
                    wk8 = w_p.tile([128, 4096], fp8, tag="wk8", name="wk8")
                    wvt = w_p.tile([128, NC_ * 512], bf16, tag="wvt", name="wvt")
                    nc.sync.dma_start(wq8[:], WQ8[:])
                    wq8v = wq8[:].rearrange("p (g i m) -> p g i m", g=4, i=2)
                    wk8v = wk8[:].rearrange("p (g i m) -> p g i m", g=4, i=2)

                    for tb in range(NTB):
                        t0 = tb * TB
                        x8t = x8_p.tile([128, 8, TB], fp8, tag="x8", name="x8")
                        nc.sync.dma_start(
                            x8t[:], X8[:, t0:t0 + TB].rearrange(
                                "(c p) t -> p c t", p=128))
                        if tb == 0:
                            nc.sync.dma_start(wk8[:], WK8[:])
                            nc.sync.dma_start(wvt[:], WV[:])
                        xtt = xt_p.tile([128, 8, TB], bf16, tag="xt", name="xt")
                        nc.sync.dma_start(
                            xtt[:], XT[:, t0:t0 + TB].rearrange(
                                "(c p) t -> p c t", p=128))
                        ett = e_p.tile([128, HL, 4, K], bf16, tag="et", name="et")
                        nc.sync.dma_start(
                            ett[:].rearrange("p h s r -> p (h s r)"),
                            ED[:, tb * HL * 4 * K:(tb + 1) * HL * 4 * K])
                        if tb == 0:
                            nc.sync.dma_start(wpt[:], WPT[:])
                        x8v = x8t[:].rearrange("p (g i) t -> p g i t", g=4)

                        # Q projection (fp8 DoubleRow) -> resident qT bf16
                        for m in range(4):
                            psq = psq_p.tile([128, TB], f32, tag="psq", name="psq")
                            for g in range(4):
                                nc.tensor.matmul(
                                    psq[:],
                                    wq8v[:, g, :, m * 128:(m + 1) * 128],
                                    x8v[:, g, :, :],
                                    start=(g == 0), stop=(g == 3), perf_mode=DR)
                            if m % 2 == 0:
                                nc.scalar.copy(qres[m][:, t0:t0 + TB], psq[:])
                            else:
                                nc.vector.tensor_copy(qres[m][:, t0:t0 + TB], psq[:])

                        # K (fp8 DoubleRow) + V (bf16) -> packed kv tiles:
                        # even head h: cols [h*128: k(64) | v(64)], odd head: [v | k]
                        kvsb = []
                        for sub in range(4):
                            psk = psk_p.tile([128, 512], f32, tag="psk", name="psk")
                            psv = psv_p.tile([128, 512], f32, tag="psv", name="psv")
                            for g in range(4):
                                nc.tensor.matmul(
                                    psk[:],
                                    x8v[:, g, :, sub * 128:(sub + 1) * 128],
                                    wk8v[:, g, :, :],
                                    start=(g == 0), stop=(g == 3), perf_mode=DR)
                            for c in range(NC_):
                                nc.tensor.matmul(
                                    psv[:],
                                    xtt[:, c, sub * 128:(sub + 1) * 128],
                                    wvt[:, c * 512:(c + 1) * 512],
                                    start=(c == 0), stop=(c == NC_ - 1))
                            kvt = kv_p.tile([128, 1024], bf16, tag=f"kv{sub}", name=f"kv{sub}")
                            kv4 = kvt[:].rearrange("p (hp x s) -> p hp x s", hp=4, x=4, s=HS)
                            psk4 = psk[:].rearrange("p (hp e s) -> p hp e s", hp=4, e=2, s=HS)
                            psv4 = psv[:].rearrange("p (hp e s) -> p hp e s", hp=4, e=2, s=HS)
                            nc.scalar.copy(kv4[:, :, 0, :], psk4[:, :, 0, :])
                            nc.vector.tensor_copy(kv4[:, :, 1, :], psv4[:, :, 0, :])
                            nc.scalar.copy(kv4[:, :, 2, :], psv4[:, :, 1, :])
                            nc.vector.tensor_copy(kv4[:, :, 3, :], psk4[:, :, 1, :])
                            kvsb.append(kvt)

                        # kpT/vpT accumulation; even h -> rows [kpT; vpT], odd -> [vpT; kpT]
                        for h in range(HL):
                            pskv = pskv_p.tile([128, K], f32, tag="pskv", name="pskv")
                            for sub in range(4):
                                nc.tensor.matmul(pskv[:], kvsb[sub][:, h * 128:(h + 1) * 128],
                                                 ett[:, h, sub, :],
                                                 start=(sub == 0), stop=(sub == 3))
                            if tb == 0:
                                nc.vector.tensor_copy(kvacc[h][:].bitcast(f32r), pskv[:])
                            else:
                                nc.vector.tensor_tensor(kvacc[h][:].bitcast(f32r), kvacc[h][:],
                                                        pskv[:], op=mybir.AluOpType.add)

                    # phase 1.5: kpb bf16 (other head's rows zeroed); vp via
                    # transpose into vpz half-columns (other half zeroed)
                    for h in range(HL):
                        lo = (h % 2 == 0)   # kpT rows 0:64 if even head, else 64:128
                        if lo:
                            nc.scalar.copy(kpb[h][0:64, :], kvacc[h][0:64, :])
                            nc.gpsimd.memset(kpb[h][64:128, :], 0.0)
                            nc.gpsimd.memset(vpz[h][:, :, 64:128], 0.0)
                        else:
                            nc.scalar.copy(kpb[h][64:128, :], kvacc[h][64:128, :])
                            nc.gpsimd.memset(kpb[h][0:64, :], 0.0)
                            nc.gpsimd.memset(vpz[h][:, :, 0:64], 0.0)
                        for j in range(2):
                            psvp = pskv_p.tile([128, K], f32, tag="pskv", name="psvp")
                            nc.tensor.transpose(psvp[:, 0:128].bitcast(f32r),
                                                kvacc[h][:, j * 128:(j + 1) * 128].bitcast(f32r),
                                                ident[:])
                            vcols = psvp[:, 64:128] if lo else psvp[:, 0:64]
                            dst = vpz[h][:, j, 0:64] if lo else vpz[h][:, j, 64:128]
                            if j == 0:
                                nc.scalar.copy(dst, vcols)
                            else:
                                nc.vector.tensor_copy(dst, vcols)

            # ================= PHASE 2 =================
            if phases & 2:
                with ExitStack() as s2:
                    eT_p = s2.enter_context(tc.tile_pool(name="eT", bufs=8))
                    rec_p = s2.enter_context(tc.tile_pool(name="rec", bufs=2))
                    at_p = s2.enter_context(tc.tile_pool(name="at", bufs=2))
                    out_p = s2.enter_context(tc.tile_pool(name="outp", bufs=3))
                    psS_p = s2.enter_context(tc.tile_pool(name="psS", bufs=2, space="PSUM"))
                    psA_p = s2.enter_context(tc.tile_pool(name="psA", bufs=2, space="PSUM"))
                    psZ_p = s2.enter_context(tc.tile_pool(name="psZ", bufs=2, space="PSUM"))
                    psp_p = s2.enter_context(tc.tile_pool(name="psp", bufs=2, space="PSUM"))

                    for tb in range(NTB):
                        t0 = tb * TB
                        attT = [at_p.tile([128, TB], f32r, tag=f"attT{p}", name=f"attT{p}")
                                for p in range(4)]
                        for pr in range(4):
                            psATT = psA_p.tile([128, TB], f32, tag="psATT", name="psATT")
                            psZ = psZ_p.tile([128, TB], f32, tag="psZ", name="psZ")
                            eTs = [[None, None], [None, None]]
                            for h01 in range(2):
                                h = 2 * pr + h01
                                for j in range(2):
                                    psS = psS_p.tile([128, TB], f32, tag="psS", name="psS")
                                    nc.tensor.matmul(
                                        psS[:],
                                        kpb[h][:, j * 128:(j + 1) * 128],
                                        qres[pr][:, t0:t0 + TB],
                                        start=True, stop=True)
                                    if tb == 0:
                                        nc.vector.tensor_tensor(
                                            psS[:], psS[:], masksT[j][:],
                                            op=mybir.AluOpType.add)
                                    eT = eT_p.tile([128, TB], bf16, tag="eT", name="eT")
                                    nc.scalar.activation(eT[:], psS[:], AF.Exp,
                                                         scale=float(SCALE))
                                    eTs[h01][j] = eT
                            # att numerators for both heads accumulate into one
                            # bank: rows 0:64 even head, 64:128 odd head
                            for i, (h01, j) in enumerate(
                                    ((0, 0), (0, 1), (1, 0), (1, 1))):
                                nc.tensor.matmul(
                                    psATT[:], vpz[2 * pr + h01][:, j, :],
                                    eTs[h01][j][:],
                                    start=(i == 0), stop=(i == 3))
                            # z broadcast rows: 0:64 = z_even, 64:128 = z_odd
                            for i, (h01, j) in enumerate(
                                    ((0, 0), (0, 1), (1, 0), (1, 1))):
                                nc.tensor.matmul(
                                    psZ[:], zsel[h01][:], eTs[h01][j][:],
                                    start=(i == 0), stop=(i == 3))
                            zrec = rec_p.tile([128, TB], f32, tag="zrec", name="zrec")
                            nc.vector.reciprocal(zrec[:], psZ[:])
                            nc.vector.tensor_tensor(attT[pr][:], psATT[:], zrec[:],
                                                    op=mybir.AluOpType.mult)

                        # output projection for this t-block (2 subs per DMA)
                        for sp in range(2):
                            outsb = out_p.tile([128, 2, C], f32, tag="outsb", name="outsb")
                            for si in range(2):
                                sub = sp * 2 + si
                                for n in range(2):
                                    psp = psp_p.tile([128, 512], f32, tag="psp", name="psp")
                                    for ci in range(4):
                                        nc.tensor.matmul(
                                            psp[:],
                                            attT[ci][:, sub * 128:(sub + 1) * 128],
                                            wpt[:, ci * C + n * 512:ci * C + (n + 1) * 512],
                                            start=(ci == 0), stop=(ci == 3))
                                    if n == 0:
                                        nc.scalar.copy(outsb[:, si, 0:512], psp[:])
                                    else:
                                        nc.vector.tensor_copy(outsb[:, si, 512:1024], psp[:])
                            row = t0 + sp * 256
                            nc.sync.dma_start(
                                O[row:row + 256, :].rearrange("(s p) c -> p s c", p=128),
                                outsb[:])

        if timing:
            dpool = top.enter_context(tc.tile_pool(name="dummy", bufs=1))
            dt_ = dpool.tile([128, 128], f32, tag="dummy", name="dummy")
            nc.sync.dma_start(dt_[:], DIN[:])
            nc.sync.dma_start(DOUT[:], dt_[:])

    nc.finalize()
    return nc


_NC_CACHE = {}


def _get_program(phases=3):
    if phases not in _NC_CACHE:
        _NC_CACHE[phases] = _build_program(phases)
    return _NC_CACHE[phases]


def _pack_w(w_core):
    """[C, 512] -> [128, 8*512] with chunk c at cols c*512."""
    return np.ascontiguousarray(
        w_core.reshape(NC_, 128, 512).transpose(1, 0, 2).reshape(128, NC_ * 512))


def _pack_w8(w_core):
    """[C, 512] f32 -> [128, 4096] e4m3, layout [p, g, i, m], scaled by WS."""
    import ml_dtypes
    a = (np.ascontiguousarray(w_core, np.float32) * WS).reshape(4, 2, 128, 512)
    a = a.transpose(2, 0, 1, 3).reshape(128, 4096)
    return np.ascontiguousarray(a).astype(ml_dtypes.float8_e4m3)


def _make_in_maps(x, WQ, WK, WV, E, Wp):
    import ml_dtypes
    xr = np.transpose(np.asarray(x), (0, 2, 1))                    # [B, C, T] f32
    xb = xr.astype(ml_dtypes.bfloat16)
    x8 = xr.astype(ml_dtypes.float8_e4m3)
    wq_full = np.transpose(np.asarray(WQ), (1, 0, 2))              # [C, H, HS] f32
    wk_full = np.transpose(np.asarray(WK), (1, 0, 2))
    wv_full = np.transpose(np.asarray(WV), (1, 0, 2)).astype(ml_dtypes.bfloat16)
    er = np.asarray(E).astype(ml_dtypes.bfloat16)                  # [H, B, T, K]

    def _pack_e(e_core):
        """[HL, T, K] -> [128, NTB*HL*4*K]: (tb, h, s) blocks of [128, K]."""
        a = e_core.reshape(HL, NTB, 4, 128, K)        # [h, tb, s, p, r]
        a = a.transpose(3, 1, 0, 2, 4)                # [p, tb, h, s, r]
        return np.ascontiguousarray(a.reshape(128, NTB * HL * 4 * K))
    wpt_full = to_f32r(np.ascontiguousarray(np.asarray(Wp).T))     # [C_in, C_out]

    mskT = np.zeros((2, 128, TB), np.float32)
    for j in range(2):
        r_idx = j * 128 + np.arange(128)[:, None]
        mskT[j] = np.where(r_idx <= np.arange(TB)[None, :], 0.0, -1e30)
    idn = np.eye(128, dtype=np.float32)

    in_maps = []
    for core in range(8):
        b, g = core // 2, core % 2
        hs = slice(g * HL, (g + 1) * HL)
        wpt_core = wpt_full[g * 512:(g + 1) * 512, :]              # [512, 1024]
        wpt_packed = np.ascontiguousarray(
            wpt_core.reshape(4, 128, C).transpose(1, 0, 2).reshape(128, 4 * C))
        in_maps.append({
            "X8": np.ascontiguousarray(x8[b]),
            "XT": np.ascontiguousarray(xb[b]),
            "WQ8": _pack_w8(np.ascontiguousarray(wq_full[:, hs, :]).reshape(C, HL * HS)),
            "WK8": _pack_w8(np.ascontiguousarray(wk_full[:, hs, :]).reshape(C, HL * HS)),
            "WV": _pack_w(np.ascontiguousarray(wv_full[:, hs, :]).reshape(C, HL * HS)),
            "ED": _pack_e(np.ascontiguousarray(er[hs, b])),
            "WPT": wpt_packed,
            "MSKT": mskT,
            "IDN": idn,
        })
    return in_maps


def _run(x, WQ, WK, WV, E, Wp, bp, trace=False):
    nc = _get_program()
    in_maps = _make_in_maps(x, WQ, WK, WV, E, Wp)
    kw = {}
    if trace:
        kw = dict(trace=True, trace_cores=[0])
    res = run_bass_kernel_spmd(nc, in_maps, list(range(8)), **kw)
    out = np.zeros((B, T, C), np.float32)
    for b in range(B):
        out[b] = res.results[2 * b]["O"] + res.results[2 * b + 1]["O"]
    out += np.asarray(bp, np.float32)[None, None, :]
    return out, res


def kernel(x, WQ, WK, WV, E, Wp, bp):
    out, _ = _run(x, WQ, WK, WV, E, Wp, bp, trace=False)
    return out


def kernel_traced(x, WQ, WK, WV, E, Wp, bp):
    out, res = _run(x, WQ, WK, WV, E, Wp, bp, trace=True)
    return out, res
